# revision 40
# baseline (speedup 1.0000x reference)
"""Multi-head attention kernel for Trainium2, 8 NeuronCores.

Sharding: DP4 (batch) x TP2 (heads). Core c handles batch c//2 with head
half c%2 (8 of 16 heads). Each core computes a partial output
(its heads' contribution to the O-projection); the host sums the two
partials per batch and adds bo.

Key optimizations over a straightforward implementation:
  * Masked keys (v_mask == 0, ~half) contribute exactly zero to the
    reference output (softmax weight exp(-1e12/8) == 0 in f32), so the
    host compacts k/v/mask to surviving rows padded to SK (multiple of
    128). All key-side work shrinks by ~SK/2048.
  * Software-pipelined emission: attention (scores -> exp -> AV) slots
    are interleaved into the Q/V projection passes so the Tensor engine
    stays continuously busy (keeps its DVFS p-state at full clock) while
    the Activation engine chews through the exps.
  * vwm carries the mask replicated in columns 64..127 of each head's
    block, so the AV matmul broadcasts the softmax denominator into
    PSUM partitions 64..127 for free; normalization is then just
    copy -> reciprocal_approx_fast -> multiply on DVE.
  * Q/K biases ride the PSUM->SBUF copies on the Scalar engine
    (Identity activation with per-partition bias); transposes run as
    float32r (1.5 cycles/row).
"""

import numpy as np

import concourse.bass as bass
import concourse.bacc as bacc
import concourse.mybir as mybir
import concourse.tile as tile
from concourse.bass_utils import run_bass_kernel_spmd
from concourse.masks import make_identity

mdt = mybir.dt
F32 = mdt.float32
F32R = mdt.float32r
BF16 = mdt.bfloat16

S = 2048          # query sequence length
D = 1024          # model dim
HL = 8            # heads per core (local)
DH = HL * 64      # local projection width (512)
JB = 512          # phase-1 j-block
NCT = D // 128    # 8 contraction tiles
NJC = S // 512    # 4 phase-2 j-chunks
NJT = S // 128    # 16 j tiles


def r32(ap):
    return ap.bitcast(F32R)


def build_nc(SK, debug=False):
    NKT = SK // 128
    # k/v phase-1 blocks: (start row, valid 128-row subtiles)
    KBLK = []
    r = 0
    while r < SK:
        KBLK.append((r, min(4, (SK - r) // 128)))
        r += 512
    QBLK = [(i * JB, 4) for i in range(S // JB)]
    # phase-2 kt groups (pairs; possibly one ragged single at the end)
    GROUPS = [list(range(g, min(g + 2, NKT))) for g in range(0, NKT, 2)]
    NG = len(GROUPS)
    LAG = 2           # AV trails exp by this many groups
    SLOT_COST = 1500  # PE-cycles of projection fill between slot emissions

    nc = bacc.Bacc("TRN2", target_bir_lowering=False, debug=False, num_devices=8)

    xq = nc.dram_tensor("xq", [S, D], F32, kind="ExternalInput")
    xk = nc.dram_tensor("xk", [SK, D], F32, kind="ExternalInput")
    xv = nc.dram_tensor("xv", [SK, D], F32, kind="ExternalInput")
    msk = nc.dram_tensor("mask", [SK, 1], F32, kind="ExternalInput")
    wq_d = nc.dram_tensor("wq", [D, DH], F32, kind="ExternalInput")
    wk_d = nc.dram_tensor("wk", [D, DH], F32, kind="ExternalInput")
    wv_d = nc.dram_tensor("wv", [D, DH], F32, kind="ExternalInput")
    bq_d = nc.dram_tensor("bq", [1, DH], F32, kind="ExternalInput")
    bk_d = nc.dram_tensor("bk", [1, DH], F32, kind="ExternalInput")
    bv_d = nc.dram_tensor("bv", [1, DH], F32, kind="ExternalInput")
    wo_d = nc.dram_tensor("wo", [DH, D], F32, kind="ExternalInput")
    out_d = nc.dram_tensor("out", [S, D], F32, kind="ExternalOutput")
    if debug:
        dq_d = nc.dram_tensor("dq", [128, S], BF16, kind="ExternalOutput")
        dk_d = nc.dram_tensor("dk", [128, SK], BF16, kind="ExternalOutput")
        dv_d = nc.dram_tensor("dv", [128, HL * 128], BF16, kind="ExternalOutput")
        doT_d = nc.dram_tensor("doT", [128, S], F32, kind="ExternalOutput")
        dden_d = nc.dram_tensor("dden", [1, 1024], F32, kind="ExternalOutput")

    with tile.TileContext(nc) as tc:
        with (
            tc.tile_pool(name="pers", bufs=1) as pers,
            tc.tile_pool(name="psA", bufs=2, space="PSUM") as psA,
            tc.tile_pool(name="psS", bufs=2, space="PSUM") as psS,
            tc.tile_pool(name="po", bufs=2, space="PSUM") as po,
            tc.tile_pool(name="ph1c", bufs=1) as ph1c,
            tc.tile_pool(name="w3", bufs=1) as w3p,
            tc.tile_pool(name="x_in", bufs=5) as x_in,
            tc.tile_pool(name="xT", bufs=8) as xTp,
            tc.tile_pool(name="expA", bufs=6) as expp,
            tc.tile_pool(name="bsbp", bufs=1) as bsbp,
            tc.tile_pool(name="outsb", bufs=2) as outp,
        ):
            # --- persistent constants / activations ---
            ones = pers.tile([1, 512], F32, tag="ones")
            m_sb = pers.tile([128, NKT], F32, tag="m_sb")
            wo = pers.tile([128, 4, D], F32, tag="wo")
            qwT = [pers.tile([128, S], BF16, tag=f"qwT{t}", name=f"qwT{t}") for t in range(4)]
            # kwT[dh]: partitions 0:64 = head 2dh keys, 64:128 = head 2dh+1
            kwT = [pers.tile([128, SK], BF16, tag=f"kwT{t}", name=f"kwT{t}") for t in range(4)]
            # vwm[kt]: cols h*128..h*128+63 = vw_h * m, cols h*128+64..+127
            # all = m (AV then replicates the denominator into output
            # partitions 64..127 for free)
            vwm = [pers.tile([128, HL * 128], BF16, tag=f"vwm{t}", name=f"vwm{t}") for t in range(NKT)]
            oTn = [pers.tile([128, S], F32, tag=f"oTn{t}", name=f"oTn{t}") for t in range(4)]
            if debug:
                dden_sb = pers.tile([1, 1024], F32, tag="dden_sb")

            Exp = mybir.ActivationFunctionType.Exp

            # ---------- phase-2 slot machinery ----------
            deferred = []     # pend steps + o-proj units, drained 1/slot
            av_fifo = []      # (p_o, h, gi, e2, gh, jc)
            slots = [(jc, h, gi) for jc in range(NJC) for h in range(HL)
                     for gi in range(NG)]
            st2 = {"i": 0, "p_o": None, "credit": 0, "s2_done_gh": -1}
            q_emitted = set()
            v_emitted = set()

            def drain_one():
                if deferred:
                    deferred.pop(0)()

            def pend_steps(p_o_, t4_, poff_, jc_, gh_):
                st = {}

                def s0():
                    # p_o[64:128] holds 64 replicated copies of the softmax
                    # denominator; approx reciprocal needs SBUF input
                    bsb = bsbp.tile([64, 512], F32, tag="bsb", name="bsb")
                    nc.vector.tensor_copy(bsb[:], p_o_[64:128, :])
                    st["bsb"] = bsb
                    if debug and t4_ == 0 and poff_ == 0 and jc_ == 0:
                        nc.vector.tensor_copy(dden_sb[:, 0:512], p_o_[64:65, :])

                def s1():
                    rsb = bsbp.tile([64, 512], F32, tag="rsb", name="rsb")
                    nc.vector.reciprocal_approx_fast(rsb[:], st["bsb"][:])
                    st["rsb"] = rsb
                    if debug and t4_ == 0 and poff_ == 0 and jc_ == 0:
                        nc.vector.tensor_copy(dden_sb[:, 512:1024], rsb[0:1, :])

                def s2():
                    nc.vector.tensor_mul(
                        r32(oTn[t4_][poff_:poff_ + 64, jc_ * 512:(jc_ + 1) * 512]),
                        p_o_[0:64, :],
                        st["rsb"][:],
                    )
                    st2["s2_done_gh"] = max(st2["s2_done_gh"], gh_)

                return [s0, s1, s2]

            def av_pop():
                p_o_, h_, gi_, e2_, gh_, jc_ = av_fifo.pop(0)
                for i, kt in enumerate(GROUPS[gi_]):
                    nc.tensor.matmul(
                        p_o_[:],
                        vwm[kt][:, h_ * 128:(h_ + 1) * 128],
                        e2_[:, i * 512:(i + 1) * 512],
                        start=(kt == 0),
                        stop=(kt == NKT - 1),
                    )
                if gi_ == NG - 1:
                    # the head's accumulation is fully emitted: queue its
                    # normalization chain
                    deferred.extend(
                        pend_steps(p_o_, h_ // 2, (h_ % 2) * 64, jc_, gh_)
                    )

            def av_poppable():
                if not av_fifo:
                    return False
                gi_ = av_fifo[0][2]
                return all(kt in v_emitted for kt in GROUPS[gi_])

            def oproj_unit(jt, mh):
                def f():
                    pm = psS.tile([128, 1024], F32, tag="ps2", name="pm")
                    for dt_ in range(4):
                        nc.tensor.matmul(
                            pm[:, 0:512],
                            r32(oTn[dt_][:, jt * 128:(jt + 1) * 128]),
                            r32(wo[:, dt_, mh * 512:(mh + 1) * 512]),
                            start=(dt_ == 0),
                            stop=(dt_ == 3),
                            skip_group_check=True,
                        )
                    o_sb = outp.tile([128, 512], F32, tag="o_sb", name="o_sb")
                    nc.vector.tensor_copy(o_sb[:], pm[:, 0:512])
                    nc.sync.dma_start(
                        out_d[jt * 128:(jt + 1) * 128, mh * 512:(mh + 1) * 512],
                        o_sb[:],
                    )
                return f

            def emit_slot():
                i = st2["i"]
                if i >= len(slots):
                    return False
                jc, h, gi = slots[i]
                if jc not in q_emitted:
                    return False
                if len(av_fifo) >= 5 and not av_poppable():
                    return False
                gh = jc * HL + h
                if gi == 0:
                    # before reusing the h-2 head's PSUM accumulator, its AV
                    # groups and normalization reads must all be emitted
                    while av_fifo and av_fifo[0][4] <= gh - 2:
                        if not av_poppable():
                            return False
                        av_pop()
                    while st2["s2_done_gh"] < gh - 2:
                        if not deferred:
                            return False
                        drain_one()
                st2["i"] = i + 1
                t4, poff = h // 2, (h % 2) * 64
                if gi == 0:
                    st2["p_o"] = po.tile([128, 512], F32, tag="po", name="p_o")
                p_o = st2["p_o"]
                grp = GROUPS[gi]
                gw = len(grp) * 512
                ps2 = psS.tile([128, 1024], F32, tag="ps2", name="ps2")
                for i2, kt in enumerate(grp):
                    nc.tensor.matmul(
                        ps2[:, i2 * 512:(i2 + 1) * 512],
                        kwT[t4][poff:poff + 64, kt * 128:(kt + 1) * 128],
                        qwT[t4][poff:poff + 64, jc * 512:(jc + 1) * 512],
                        start=True,
                        stop=True,
                        skip_group_check=True,
                    )
                e2 = expp.tile([128, 1024], BF16, tag="e2", name="e2")
                nc.scalar.activation(e2[:, :gw], ps2[:, :gw], Exp, scale=0.125)
                av_fifo.append((p_o, h, gi, e2, gh, jc))
                while len(av_fifo) > LAG and av_poppable():
                    av_pop()
                drain_one()
                if gi == NG - 1 and h == 1 and jc > 0:
                    for jt in range((jc - 1) * 4, (jc - 1) * 4 + 4):
                        for mh in range(2):
                            deferred.append(oproj_unit(jt, mh))
                return True

            def tick(cycles):
                st2["credit"] += cycles
                while st2["credit"] >= SLOT_COST:
                    if not emit_slot():
                        st2["credit"] = min(st2["credit"], SLOT_COST)
                        break
                    st2["credit"] -= SLOT_COST

            # ---------- phase 1: transposes + projections ----------
            def phase1(x_dram, w_sb, kind, blocks, ident,
                       bias_col=None, bv_sb=None, first_cb=None):
                for bi, (row0, vjs) in enumerate(blocks):
                    w = vjs * 128
                    xi = []
                    for t in range(vjs):
                        xt_ = x_in.tile([128, D], F32, tag="xi", name="xi")
                        nc.sync.dma_start(
                            r32(xt_[:]),
                            r32(x_dram[row0 + t * 128: row0 + (t + 1) * 128, :]),
                        )
                        xi.append(xt_)
                    if bi == 0 and first_cb is not None:
                        first_cb()
                    xT = []
                    for ct in range(NCT):
                        pt = psA.tile([128, 512], F32, tag="ps", name="pt")
                        for js in range(vjs):
                            nc.tensor.matmul(
                                r32(pt[:, js * 128:(js + 1) * 128]),
                                r32(xi[js][:, ct * 128:(ct + 1) * 128]),
                                ident[:],
                                is_transpose=True,
                                skip_group_check=True,
                            )
                        xt_sb = xTp.tile([128, 512], F32, tag="xT", name="xt_sb")
                        nc.vector.tensor_copy(r32(xt_sb[:, :w]), pt[:, :w])
                        xT.append(xt_sb)
                        if ct % 2 == 1:
                            tick(vjs * 128 * 3)

                    if kind in ("q", "k"):
                        # out (dh, j): stationary = W chunk, moving = xT (the
                        # ragged last k block reads junk past w; only valid
                        # columns are copied out)
                        for dh in range(4):
                            pq = psA.tile([128, 512], F32, tag="ps", name="pq")
                            for ct in range(NCT):
                                nc.tensor.matmul(
                                    pq[:, :JB],
                                    r32(w_sb[:, ct, dh * 128:(dh + 1) * 128]),
                                    r32(xT[ct][:]),
                                    start=(ct == 0),
                                    stop=(ct == NCT - 1),
                                    skip_group_check=True,
                                )
                            if kind == "q":
                                nc.scalar.add(
                                    qwT[dh][:, row0:row0 + w], pq[:, :w],
                                    bias_col[:, dh:dh + 1],
                                )
                            else:
                                nc.scalar.add(
                                    kwT[dh][0:64, row0:row0 + w], pq[0:64, :w],
                                    bias_col[0:64, dh:dh + 1],
                                )
                                nc.scalar.add(
                                    kwT[dh][64:128, row0:row0 + w], pq[64:128, :w],
                                    bias_col[64:128, dh:dh + 1],
                                )
                            tick(NCT * 512)
                        if kind == "q":
                            q_emitted.add(row0 // JB)
                    else:
                        for js in range(vjs):
                            jt = row0 // 128 + js
                            pv = psA.tile([128, 512], F32, tag="ps", name="pv")
                            for ct in range(NCT):
                                nc.tensor.matmul(
                                    pv[:, :512],
                                    r32(xT[ct][:, js * 128:(js + 1) * 128]),
                                    r32(w_sb[:, ct, :]),
                                    start=(ct == 0),
                                    stop=False,
                                    skip_group_check=True,
                                )
                            nc.tensor.matmul(
                                pv[:, :512],
                                r32(ones[0:1, 0:128]),
                                r32(bv_sb[0:1, :]),
                                start=False,
                                stop=True,
                                skip_group_check=True,
                            )
                            for h in range(HL):
                                nc.vector.tensor_scalar_mul(
                                    vwm[jt][:, h * 128: h * 128 + 64],
                                    pv[:, h * 64:(h + 1) * 64],
                                    m_sb[:, jt: jt + 1],
                                )
                                nc.vector.tensor_scalar_add(
                                    vwm[jt][:, h * 128 + 64: (h + 1) * 128],
                                    zsb[:],
                                    m_sb[:, jt: jt + 1],
                                )
                            v_emitted.add(jt)
                            tick(NCT * 512 + 512)

            # ---------- init ----------
            ident_f = ph1c.tile([128, 128], F32, tag="ident_f")
            make_identity(nc, ident_f[:])
            ident = ph1c.tile([128, 128], F32R, tag="ident")
            nc.vector.tensor_copy(ident[:], r32(ident_f[:]))
            nc.sync.dma_start(
                m_sb[:], msk.ap().rearrange("(kt p) one -> p (kt one)", p=128)
            )
            ones_raw = ph1c.tile([1, 512], F32, tag="ones_raw")
            nc.vector.memset(ones_raw[:], 1.0)
            nc.vector.tensor_copy(r32(ones[:]), ones_raw[:])
            zsb = ph1c.tile([128, 64], F32, tag="zsb")
            nc.vector.memset(zsb[:], 0.0)
            # touch the exp table so ACT_TABLE_LOAD is off the critical path
            warm = ph1c.tile([1, 4], F32, tag="warm")
            nc.gpsimd.memset(warm[0:1, 0:2], 0.0)
            nc.scalar.activation(
                warm[0:1, 2:4], warm[0:1, 0:2], Exp
            )

            wq = w3p.tile([128, NCT, DH], F32, tag="wA", name="wq")
            wk = w3p.tile([128, NCT, DH], F32, tag="wB", name="wk")
            wv = w3p.tile([128, NCT, DH], F32, tag="wC", name="wv")
            bq = ph1c.tile([128, 4], F32, tag="bq")
            bk = ph1c.tile([128, 4], F32, tag="bk")
            bv = ph1c.tile([1, DH], F32, tag="bv")

            def load_qk_weights():
                for ct in range(NCT):
                    nc.sync.dma_start(r32(wk[:, ct, :]), r32(wk_d[ct * 128:(ct + 1) * 128, :]))
                    nc.sync.dma_start(r32(wq[:, ct, :]), r32(wq_d[ct * 128:(ct + 1) * 128, :]))
                nc.sync.dma_start(
                    bq[:], bq_d.ap().rearrange("one (c p) -> p (c one)", p=128)
                )
                nc.sync.dma_start(
                    bk[:], bk_d.ap().rearrange("one (c p) -> p (c one)", p=128)
                )
                nc.sync.dma_start(r32(bv[:]), r32(bv_d[:, :]))

            # ---------- emission schedule ----------
            # K fully (scores need all of kwT), then Q block 0 (unlocks
            # jc=0 slots), then V and remaining Q with attention slots
            # interleaved via tick().
            phase1(xk, wk, "k", KBLK, ident, bias_col=bk, first_cb=load_qk_weights)
            phase1(xq, wq, "q", QBLK[:1], ident, bias_col=bq)
            for ct in range(NCT):
                nc.sync.dma_start(r32(wv[:, ct, :]), r32(wv_d[ct * 128:(ct + 1) * 128, :]))
            for dt_ in range(4):
                nc.sync.dma_start(r32(wo[:, dt_, :]), r32(wo_d[dt_ * 128:(dt_ + 1) * 128, :]))
            phase1(xv, wv, "v", KBLK, ident, bv_sb=bv)
            phase1(xq, wq, "q", QBLK[1:], ident, bias_col=bq)

            # ---------- drain remaining slots ----------
            while st2["i"] < len(slots):
                if not emit_slot():
                    if av_poppable():
                        av_pop()
                    else:
                        break
            while av_fifo:
                av_pop()
            while deferred:
                drain_one()
            for jt in range((NJC - 1) * 4, (NJC - 1) * 4 + 4):
                for mh in range(2):
                    oproj_unit(jt, mh)()

            if debug:
                nc.sync.dma_start(dq_d[:, :], qwT[0][:])
                nc.sync.dma_start(dk_d[:, :], kwT[0][:])
                nc.sync.dma_start(dv_d[:, :], vwm[0][:])
                nc.sync.dma_start(doT_d[:, :], oTn[0][:])
                nc.sync.dma_start(dden_d[:, :], dden_sb[:])

    nc.compile()
    return nc


_NC = {}


def _get_nc(SK):
    if SK not in _NC:
        _NC[SK] = build_nc(SK)
    return _NC[SK]


def _compact(k, v, v_mask):
    """Drop masked-out key rows (exactly zero contribution), pad to a
    multiple of 128."""
    B = k.shape[0]
    counts = [int(np.asarray(v_mask[b]).astype(np.int64).sum()) for b in range(B)]
    SK = max(128, int(-(-max(counts) // 128) * 128))
    kc = np.zeros((B, SK, D), dtype=np.float32)
    vc = np.zeros((B, SK, D), dtype=np.float32)
    mc = np.zeros((B, SK), dtype=np.float32)
    for b in range(B):
        idx = np.nonzero(np.asarray(v_mask[b]).astype(np.int64))[0]
        n = len(idx)
        kc[b, :n] = np.asarray(k[b])[idx]
        vc[b, :n] = np.asarray(v[b])[idx]
        mc[b, :n] = 1.0
    return kc, vc, mc, SK


def make_in_maps(q, k, v, v_mask, Wq, bq, Wk, bk, Wv, bv, Wo, bo):
    c32 = lambda a: np.ascontiguousarray(a, dtype=np.float32)
    kc, vc, mc, SK = _compact(k, v, v_mask)
    in_maps = []
    for c in range(8):
        b, t = c // 2, c % 2
        sl = slice(t * DH, (t + 1) * DH)
        in_maps.append({
            "xq": c32(q[b]),
            "xk": c32(kc[b]),
            "xv": c32(vc[b]),
            "mask": c32(mc[b].reshape(SK, 1)),
            "wq": c32(Wq[:, sl]),
            "wk": c32(Wk[:, sl]),
            "wv": c32(Wv[:, sl]),
            "bq": c32(bq[sl].reshape(1, DH)),
            "bk": c32(bk[sl].reshape(1, DH)),
            "bv": c32(bv[sl].reshape(1, DH)),
            "wo": c32(Wo[sl, :]),
        })
    return in_maps, SK


def combine(results, bo):
    out = np.empty((4, S, D), dtype=np.float32)
    for b in range(4):
        out[b] = results[2 * b]["out"] + results[2 * b + 1]["out"]
    out += np.asarray(bo, dtype=np.float32)[None, None, :]
    return out


def kernel(q, k, v, v_mask, Wq, bq, Wk, bk, Wv, bv, Wo, bo):
    in_maps, SK = make_in_maps(q, k, v, v_mask, Wq, bq, Wk, bk, Wv, bv, Wo, bo)
    nc = _get_nc(SK)
    res = run_bass_kernel_spmd(nc, in_maps, list(range(8)))
    return combine(res.results, bo)


# revision 41
# speedup vs baseline: 1.2366x; 1.2366x over previous
"""Multi-head attention kernel for Trainium2, 8 NeuronCores.

Sharding: DP4 (batch) x TP2 (heads). Core c handles batch c//2 with head
half c%2 (8 of 16 heads). Each core computes a partial output
(its heads' contribution to the O-projection); the host sums the two
partials per batch and adds bo.

Key optimizations over a straightforward implementation:
  * Masked keys (v_mask == 0, ~half) contribute exactly zero to the
    reference output (softmax weight exp(-1e12/8) == 0 in f32), so the
    host compacts k/v/mask to surviving rows padded to SK (multiple of
    128). All key-side work shrinks by ~SK/2048.
  * vwm carries the mask replicated in columns 64..127 of each head's
    block, so the AV matmul broadcasts the softmax denominator into
    PSUM partitions 64..127 for free; normalization is then just
    copy -> reciprocal_approx_fast -> multiply on DVE.
  * Q/K biases ride the PSUM->SBUF copies on the Scalar engine
    (Identity activation with per-partition bias); transposes run as
    float32r (1.5 cycles/row); scores contract over 64 partitions
    directly (no zero-padded halves, no startup memsets).

Per-core dataflow:
  1. PE-transpose x (tokens, c) -> xT (c, tokens) per 128x128 tile.
  2. qwT/kwT = (W.T x.T) with W chunks as stationary -> (dh, j) layout;
     vw = x W in natural (j, d') layout; V bias via a K=1 matmul.
  3. Per (head, 512-wide j-chunk): scores^T tiles (k, j) on PE (K=64),
     exp on ACT (scale=1/8 folded in, no max subtraction - scores are
     bounded for this problem), AV accumulation on PE.
  4. O-projection from the packed (d', j) attention output; DMA out.
"""

import numpy as np

import concourse.bass as bass
import concourse.bacc as bacc
import concourse.mybir as mybir
import concourse.tile as tile
from concourse.bass_utils import run_bass_kernel_spmd
from concourse.masks import make_identity

mdt = mybir.dt
F32 = mdt.float32
F32R = mdt.float32r
BF16 = mdt.bfloat16

S = 2048          # query sequence length
D = 1024          # model dim
HL = 8            # heads per core (local)
DH = HL * 64      # local projection width (512)
JB = 512          # phase-1 j-block
NCT = D // 128    # 8 contraction tiles
NJC = S // 512    # 4 phase-2 j-chunks
NJT = S // 128    # 16 j tiles


def r32(ap):
    return ap.bitcast(F32R)


def build_nc(SK, debug=False):
    NKT = SK // 128
    # k/v phase-1 blocks: (start row, valid 128-row subtiles)
    KBLK = []
    r = 0
    while r < SK:
        KBLK.append((r, min(4, (SK - r) // 128)))
        r += 512
    QBLK = [(i * JB, 4) for i in range(S // JB)]
    # phase-2 kt groups (pairs; possibly one ragged single at the end)
    GROUPS = [list(range(g, min(g + 2, NKT))) for g in range(0, NKT, 2)]
    NG = len(GROUPS)

    nc = bacc.Bacc("TRN2", target_bir_lowering=False, debug=False, num_devices=8)

    xq = nc.dram_tensor("xq", [S, D], F32, kind="ExternalInput")
    xk = nc.dram_tensor("xk", [SK, D], F32, kind="ExternalInput")
    xv = nc.dram_tensor("xv", [SK, D], F32, kind="ExternalInput")
    msk = nc.dram_tensor("mask", [SK, 1], F32, kind="ExternalInput")
    wq_d = nc.dram_tensor("wq", [D, DH], F32, kind="ExternalInput")
    wk_d = nc.dram_tensor("wk", [D, DH], F32, kind="ExternalInput")
    wv_d = nc.dram_tensor("wv", [D, DH], F32, kind="ExternalInput")
    bq_d = nc.dram_tensor("bq", [1, DH], F32, kind="ExternalInput")
    bk_d = nc.dram_tensor("bk", [1, DH], F32, kind="ExternalInput")
    bv_d = nc.dram_tensor("bv", [1, DH], F32, kind="ExternalInput")
    wo_d = nc.dram_tensor("wo", [DH, D], F32, kind="ExternalInput")
    out_d = nc.dram_tensor("out", [S, D], F32, kind="ExternalOutput")
    if debug:
        dq_d = nc.dram_tensor("dq", [128, S], BF16, kind="ExternalOutput")
        dk_d = nc.dram_tensor("dk", [128, SK], BF16, kind="ExternalOutput")
        dv_d = nc.dram_tensor("dv", [128, HL * 128], BF16, kind="ExternalOutput")
        doT_d = nc.dram_tensor("doT", [128, S], F32, kind="ExternalOutput")
        dden_d = nc.dram_tensor("dden", [1, 1024], F32, kind="ExternalOutput")

    with tile.TileContext(nc) as tc:
        with (
            tc.tile_pool(name="pers", bufs=1) as pers,
            tc.tile_pool(name="ps", bufs=3, space="PSUM") as ps,
            tc.tile_pool(name="po", bufs=2, space="PSUM") as po,
        ):
            # --- persistent constants / activations ---
            ones = pers.tile([1, 512], F32, tag="ones")
            m_sb = pers.tile([128, NKT], F32, tag="m_sb")
            wo = pers.tile([128, 4, D], F32, tag="wo")
            qwT = [pers.tile([128, S], BF16, tag=f"qwT{t}", name=f"qwT{t}") for t in range(4)]
            # kwT[dh]: partitions 0:64 = head 2dh keys, 64:128 = head 2dh+1
            kwT = [pers.tile([128, SK], BF16, tag=f"kwT{t}", name=f"kwT{t}") for t in range(4)]
            # vwm[kt]: cols h*128..h*128+63 = vw_h * m, cols h*128+64..+127
            # all = m (AV then replicates the denominator into output
            # partitions 64..127 for free)
            vwm = [pers.tile([128, HL * 128], BF16, tag=f"vwm{t}", name=f"vwm{t}") for t in range(NKT)]
            oTn = [pers.tile([128, S], F32, tag=f"oTn{t}", name=f"oTn{t}") for t in range(4)]
            if debug:
                dden_sb = pers.tile([1, 1024], F32, tag="dden_sb")

            # --- phase 1: transposes + projections (scoped pools) ---
            def phase1(x_dram, w_sb, kind, blocks, x_in, xTp, ident, zsb,
                       bias_col=None, bv_sb=None, first_cb=None):
                for bi, (row0, vjs) in enumerate(blocks):
                    w = vjs * 128
                    xi = []
                    for t in range(vjs):
                        xt_ = x_in.tile([128, D], F32, tag="xi", name="xi")
                        nc.sync.dma_start(
                            r32(xt_[:]),
                            r32(x_dram[row0 + t * 128: row0 + (t + 1) * 128, :]),
                        )
                        xi.append(xt_)
                    if bi == 0 and first_cb is not None:
                        first_cb()
                    xT = []
                    for ct in range(NCT):
                        pt = ps.tile([128, 1024], F32, tag="ps", name="pt")
                        for js in range(vjs):
                            nc.tensor.matmul(
                                r32(pt[:, js * 128:(js + 1) * 128]),
                                r32(xi[js][:, ct * 128:(ct + 1) * 128]),
                                ident[:],
                                is_transpose=True,
                                skip_group_check=True,
                            )
                        xt_sb = xTp.tile([128, 512], F32, tag="xT", name="xt_sb")
                        nc.vector.tensor_copy(r32(xt_sb[:, :w]), pt[:, :w])
                        xT.append(xt_sb)

                    if kind in ("q", "k"):
                        # out (dh, j): stationary = W chunk, moving = xT (the
                        # ragged last k block reads junk past w; only valid
                        # columns are copied out)
                        for dh in range(4):
                            pq = ps.tile([128, 1024], F32, tag="ps")
                            for ct in range(NCT):
                                nc.tensor.matmul(
                                    pq[:, :JB],
                                    r32(w_sb[:, ct, dh * 128:(dh + 1) * 128]),
                                    r32(xT[ct][:]),
                                    start=(ct == 0),
                                    stop=(ct == NCT - 1),
                                    skip_group_check=True,
                                )
                            if kind == "q":
                                nc.scalar.add(
                                    qwT[dh][:, row0:row0 + w], pq[:, :w],
                                    bias_col[:, dh:dh + 1],
                                )
                            else:
                                nc.scalar.add(
                                    kwT[dh][0:64, row0:row0 + w], pq[0:64, :w],
                                    bias_col[0:64, dh:dh + 1],
                                )
                                nc.scalar.add(
                                    kwT[dh][64:128, row0:row0 + w], pq[64:128, :w],
                                    bias_col[64:128, dh:dh + 1],
                                )
                    else:
                        for js in range(vjs):
                            jt = row0 // 128 + js
                            pv = ps.tile([128, 1024], F32, tag="ps")
                            for ct in range(NCT):
                                nc.tensor.matmul(
                                    pv[:, :512],
                                    r32(xT[ct][:, js * 128:(js + 1) * 128]),
                                    r32(w_sb[:, ct, :]),
                                    start=(ct == 0),
                                    stop=False,
                                    skip_group_check=True,
                                )
                            nc.tensor.matmul(
                                pv[:, :512],
                                r32(ones[0:1, 0:128]),
                                r32(bv_sb[0:1, :]),
                                start=False,
                                stop=True,
                                skip_group_check=True,
                            )
                            for h in range(HL):
                                nc.vector.tensor_scalar_mul(
                                    vwm[jt][:, h * 128: h * 128 + 64],
                                    pv[:, h * 64:(h + 1) * 64],
                                    m_sb[:, jt: jt + 1],
                                )
                                nc.vector.tensor_scalar_add(
                                    vwm[jt][:, h * 128 + 64: (h + 1) * 128],
                                    zsb[:],
                                    m_sb[:, jt: jt + 1],
                                )

            with (
                tc.tile_pool(name="ph1c", bufs=1) as ph1c,
                tc.tile_pool(name="w3", bufs=1) as w3p,
                tc.tile_pool(name="x_in", bufs=5) as x_in,
                tc.tile_pool(name="xT", bufs=8) as xTp,
            ):
                # identity FIRST so transposes aren't blocked
                ident_f = ph1c.tile([128, 128], F32, tag="ident_f")
                make_identity(nc, ident_f[:])
                ident = ph1c.tile([128, 128], F32R, tag="ident")
                nc.vector.tensor_copy(ident[:], r32(ident_f[:]))
                nc.sync.dma_start(
                    m_sb[:], msk.ap().rearrange("(kt p) one -> p (kt one)", p=128)
                )
                ones_raw = ph1c.tile([1, 512], F32, tag="ones_raw")
                nc.vector.memset(ones_raw[:], 1.0)
                nc.vector.tensor_copy(r32(ones[:]), ones_raw[:])
                zsb = ph1c.tile([128, 64], F32, tag="zsb")
                nc.vector.memset(zsb[:], 0.0)
                # touch the exp table so ACT_TABLE_LOAD is off the attention
                # critical path
                warm = ph1c.tile([1, 4], F32, tag="warm")
                nc.gpsimd.memset(warm[0:1, 0:2], 0.0)
                nc.scalar.activation(
                    warm[0:1, 2:4], warm[0:1, 0:2], mybir.ActivationFunctionType.Exp
                )

                wq = w3p.tile([128, NCT, DH], F32, tag="wA", name="wq")
                wk = w3p.tile([128, NCT, DH], F32, tag="wB", name="wk")
                bq = ph1c.tile([128, 4], F32, tag="bq")
                bk = ph1c.tile([128, 4], F32, tag="bk")
                bv = ph1c.tile([1, DH], F32, tag="bv")

                def load_qk_weights():
                    # emitted after the first x DMAs so compute starts early
                    for ct in range(NCT):
                        nc.sync.dma_start(r32(wq[:, ct, :]), r32(wq_d[ct * 128:(ct + 1) * 128, :]))
                        nc.sync.dma_start(r32(wk[:, ct, :]), r32(wk_d[ct * 128:(ct + 1) * 128, :]))
                    nc.sync.dma_start(
                        bq[:], bq_d.ap().rearrange("one (c p) -> p (c one)", p=128)
                    )
                    nc.sync.dma_start(
                        bk[:], bk_d.ap().rearrange("one (c p) -> p (c one)", p=128)
                    )
                    nc.sync.dma_start(r32(bv[:]), r32(bv_d[:, :]))

                phase1(xq, wq, "q", QBLK, x_in, xTp, ident, zsb,
                       bias_col=bq, first_cb=load_qk_weights)
                # wv reuses wq's slot (tag "wA"); its DMA overlaps the K pass
                wv = w3p.tile([128, NCT, DH], F32, tag="wA", name="wv")
                for ct in range(NCT):
                    nc.sync.dma_start(r32(wv[:, ct, :]), r32(wv_d[ct * 128:(ct + 1) * 128, :]))
                phase1(xk, wk, "k", KBLK, x_in, xTp, ident, zsb, bias_col=bk)
                phase1(xv, wv, "v", KBLK, x_in, xTp, ident, zsb, bv_sb=bv)

            for dt_ in range(4):
                nc.sync.dma_start(r32(wo[:, dt_, :]), r32(wo_d[dt_ * 128:(dt_ + 1) * 128, :]))

            # --- phases 2+3: attention (uniform cross-iteration pipeline) ---
            with (
                tc.tile_pool(name="expA", bufs=8) as expp,
                tc.tile_pool(name="bsbp", bufs=2) as bsbp,
                tc.tile_pool(name="outsb", bufs=2) as outp,
            ):
                Exp = mybir.ActivationFunctionType.Exp
                deferred = []   # small chunks drained one per group slot

                def drain_one():
                    if deferred:
                        deferred.pop(0)()

                def pend_steps(p_o_, t4_, poff_, jc_):
                    st = {}

                    def s0():
                        # p_o[64:128] holds 64 replicated copies of the
                        # softmax denominator; approx recip needs SBUF input
                        bsb = bsbp.tile([64, 512], F32, tag="bsb", name="bsb")
                        nc.vector.tensor_copy(bsb[:], p_o_[64:128, :])
                        st["bsb"] = bsb
                        if debug and t4_ == 0 and poff_ == 0 and jc_ == 0:
                            nc.vector.tensor_copy(dden_sb[:, 0:512], p_o_[64:65, :])

                    def s1():
                        rsb = bsbp.tile([64, 512], F32, tag="rsb", name="rsb")
                        nc.vector.reciprocal_approx_fast(rsb[:], st["bsb"][:])
                        st["rsb"] = rsb
                        if debug and t4_ == 0 and poff_ == 0 and jc_ == 0:
                            nc.vector.tensor_copy(dden_sb[:, 512:1024], rsb[0:1, :])

                    def s2():
                        nc.vector.tensor_mul(
                            r32(oTn[t4_][poff_:poff_ + 64, jc_ * 512:(jc_ + 1) * 512]),
                            p_o_[0:64, :],
                            st["rsb"][:],
                        )

                    return [s0, s1, s2]

                def av_group(p_o_, h_, gi_, e2_):
                    for i, kt in enumerate(GROUPS[gi_]):
                        nc.tensor.matmul(
                            p_o_[:],
                            vwm[kt][:, h_ * 128:(h_ + 1) * 128],
                            e2_[:, i * 512:(i + 1) * 512],
                            start=(kt == 0),
                            stop=(kt == NKT - 1),
                        )

                def oproj_unit(jt, mh):
                    def f():
                        pm = ps.tile([128, 1024], F32, tag="ps", name="pm")
                        for dt_ in range(4):
                            nc.tensor.matmul(
                                pm[:, 0:512],
                                r32(oTn[dt_][:, jt * 128:(jt + 1) * 128]),
                                r32(wo[:, dt_, mh * 512:(mh + 1) * 512]),
                                start=(dt_ == 0),
                                stop=(dt_ == 3),
                                skip_group_check=True,
                            )
                        o_sb = outp.tile([128, 512], F32, tag="o_sb", name="o_sb")
                        nc.vector.tensor_copy(o_sb[:], pm[:, 0:512])
                        nc.sync.dma_start(
                            out_d[jt * 128:(jt + 1) * 128, mh * 512:(mh + 1) * 512],
                            o_sb[:],
                        )
                    return f

                LAG = 2
                av_fifo = []    # (p_o, h, gi, e2)
                prev_pend = None
                for jc in range(NJC):
                    for h in range(HL):
                        t4, poff = h // 2, (h % 2) * 64
                        p_o = po.tile([128, 512], F32, tag="po", name="p_o")
                        for gi, grp in enumerate(GROUPS):
                            gw = len(grp) * 512
                            ps2 = ps.tile([128, 1024], F32, tag="ps", name="ps2")
                            for i2, kt in enumerate(grp):
                                nc.tensor.matmul(
                                    ps2[:, i2 * 512:(i2 + 1) * 512],
                                    kwT[t4][poff:poff + 64, kt * 128:(kt + 1) * 128],
                                    qwT[t4][poff:poff + 64, jc * 512:(jc + 1) * 512],
                                    start=True,
                                    stop=True,
                                    skip_group_check=True,
                                )
                            e2 = expp.tile([128, 1024], BF16, tag="e2", name="e2")
                            nc.scalar.activation(
                                e2[:, :gw], ps2[:, :gw], Exp, scale=0.125
                            )
                            av_fifo.append((p_o, h, gi, e2))
                            if len(av_fifo) > LAG:
                                av_group(*av_fifo.pop(0))
                            if gi == 1 and prev_pend is not None:
                                deferred.extend(pend_steps(*prev_pend))
                                prev_pend = None
                            drain_one()
                        prev_pend = (p_o, t4, poff, jc)
                        # queue O-projection for the previous j-chunk once
                        # all its heads are normalized
                        if h == 1 and jc > 0:
                            for jt in range((jc - 1) * 4, (jc - 1) * 4 + 4):
                                for mh in range(2):
                                    deferred.append(oproj_unit(jt, mh))
                # tail
                while av_fifo:
                    av_group(*av_fifo.pop(0))
                for f in pend_steps(*prev_pend):
                    f()
                while deferred:
                    drain_one()
                for jt in range((NJC - 1) * 4, (NJC - 1) * 4 + 4):
                    for mh in range(2):
                        oproj_unit(jt, mh)()

            if debug:
                nc.sync.dma_start(dq_d[:, :], qwT[0][:])
                nc.sync.dma_start(dk_d[:, :], kwT[0][:])
                nc.sync.dma_start(dv_d[:, :], vwm[0][:])
                nc.sync.dma_start(doT_d[:, :], oTn[0][:])
                nc.sync.dma_start(dden_d[:, :], dden_sb[:])

    nc.compile()
    return nc


_NC = {}


def _get_nc(SK):
    if SK not in _NC:
        _NC[SK] = build_nc(SK)
    return _NC[SK]


def _compact(k, v, v_mask):
    """Drop masked-out key rows (exactly zero contribution), pad to a
    multiple of 128."""
    B = k.shape[0]
    counts = [int(np.asarray(v_mask[b]).astype(np.int64).sum()) for b in range(B)]
    SK = max(128, int(-(-max(counts) // 128) * 128))
    kc = np.zeros((B, SK, D), dtype=np.float32)
    vc = np.zeros((B, SK, D), dtype=np.float32)
    mc = np.zeros((B, SK), dtype=np.float32)
    for b in range(B):
        idx = np.nonzero(np.asarray(v_mask[b]).astype(np.int64))[0]
        n = len(idx)
        kc[b, :n] = np.asarray(k[b])[idx]
        vc[b, :n] = np.asarray(v[b])[idx]
        mc[b, :n] = 1.0
    return kc, vc, mc, SK


def make_in_maps(q, k, v, v_mask, Wq, bq, Wk, bk, Wv, bv, Wo, bo):
    c32 = lambda a: np.ascontiguousarray(a, dtype=np.float32)
    kc, vc, mc, SK = _compact(k, v, v_mask)
    in_maps = []
    for c in range(8):
        b, t = c // 2, c % 2
        sl = slice(t * DH, (t + 1) * DH)
        in_maps.append({
            "xq": c32(q[b]),
            "xk": c32(kc[b]),
            "xv": c32(vc[b]),
            "mask": c32(mc[b].reshape(SK, 1)),
            "wq": c32(Wq[:, sl]),
            "wk": c32(Wk[:, sl]),
            "wv": c32(Wv[:, sl]),
            "bq": c32(bq[sl].reshape(1, DH)),
            "bk": c32(bk[sl].reshape(1, DH)),
            "bv": c32(bv[sl].reshape(1, DH)),
            "wo": c32(Wo[sl, :]),
        })
    return in_maps, SK


def combine(results, bo):
    out = np.empty((4, S, D), dtype=np.float32)
    for b in range(4):
        out[b] = results[2 * b]["out"] + results[2 * b + 1]["out"]
    out += np.asarray(bo, dtype=np.float32)[None, None, :]
    return out


def kernel(q, k, v, v_mask, Wq, bq, Wk, bk, Wv, bv, Wo, bo):
    in_maps, SK = make_in_maps(q, k, v, v_mask, Wq, bq, Wk, bk, Wv, bv, Wo, bo)
    nc = _get_nc(SK)
    res = run_bass_kernel_spmd(nc, in_maps, list(range(8)))
    return combine(res.results, bo)
